# revision 12
# baseline (speedup 1.0000x reference)
"""Trainium2 Bass kernel for GQA attention (32 q heads / 16 kv heads, head_dim
128, L=2048, D=4608) with RoPE, tanh softcap 50, causal mask, o_proj.

Strategy: tensor-parallel over heads across 8 NeuronCores. Core c computes
q-heads 4c..4c+3 and kv-heads 2c..2c+1 end-to-end; the host sums the 8 partial
[L, D] outputs (bf16 partials, f32 host accumulation).

v2 design (vs the two-phase baseline):
  - single software-pipelined pass over the 4 q-chunks of 512: causality lets
    attention for chunk nq start right after its projections (K/V history for
    chunks <= nq is already computed), so the Scalar engine's tanh+exp stream
    (~200us) hides under the PE's projection matmuls instead of serializing a
    separate attention phase
  - PV computed in [d, q] layout (lhsT = V tile, rhs = P^T tile, 512-wide
    streams) so every PE matmul streams >= 256 columns and LDWEIGHTS stays
    shadow-loaded; this also eliminates the per-128-column PE transposes of
    the attention output (o_proj consumes [d, q] directly)
  - softmax denominator accumulated on the otherwise-idle GpSimd engine
    (tensor_add over P^T tiles + partition_all_reduce broadcast), reciprocal
    on DVE, folded into the PV psum drain multiply
  - rope drains moved off the Scalar engine: DVE multiplies read the
    projection psum directly (cos/sin mul + rotate-half add)
  - wq/wk/wv resident; wo streamed per (chunk, j) to fit SBUF; x staged per
    chunk; outputs written bf16
"""

import numpy as np
import ml_dtypes

import concourse.bass as bass
import concourse.mybir as mybir
import concourse.tile as tile
from concourse import bacc, bass_isa

F32 = mybir.dt.float32
BF16 = mybir.dt.bfloat16
BF16_NP = ml_dtypes.bfloat16
AF = mybir.ActivationFunctionType

N_HEADS = 32
N_KV = 16
HEAD_DIM = 128
ROPE_THETA = 10000.0
SOFTCAP = 50.0
SCALE = 1.0 / 12.0  # 1/sqrt(144)
L = 2048
D = 4608
N_CORES = 8
QH = N_HEADS // N_CORES        # 4 local q heads
KVH = N_KV // N_CORES          # 2 local kv heads
KC = D // 128                  # 36 contraction chunks
NQ = L // 512                  # 4 l-chunks of 512
LT = L // 128                  # 16 l-tiles of 128


DEBUG_TAPS = False


def _emit(nc):
    xt_d = nc.dram_tensor("xt", [D, L], BF16, kind="ExternalInput")
    wqt_d = nc.dram_tensor("wqt", [D, QH * 128], BF16, kind="ExternalInput")
    wkt_d = nc.dram_tensor("wkt", [D, KVH * 128], BF16, kind="ExternalInput")
    wvt_d = nc.dram_tensor("wvt", [D, KVH * 128], BF16, kind="ExternalInput")
    wot_d = nc.dram_tensor("wot", [QH * 128, D], BF16, kind="ExternalInput")
    cost_d = nc.dram_tensor("cost", [128, L], BF16, kind="ExternalInput")
    sint_d = nc.dram_tensor("sint", [128, L], BF16, kind="ExternalInput")
    masks_d = nc.dram_tensor("masks", [4, 128, 512], BF16, kind="ExternalInput")
    out_d = nc.dram_tensor("out", [L, D], BF16, kind="ExternalOutput")
    if DEBUG_TAPS:
        qt_dbg = nc.dram_tensor("qt_dbg", [QH, 128, 512], BF16, kind="ExternalOutput")
        kt_dbg = nc.dram_tensor("kt_dbg", [KVH, 128, L], BF16, kind="ExternalOutput")
        ve_dbg = nc.dram_tensor("ve_dbg", [128, LT * 256], BF16, kind="ExternalOutput")
        at_dbg = nc.dram_tensor("at_dbg", [QH, 128, 512], BF16, kind="ExternalOutput")
        rb_dbg = nc.dram_tensor("rb_dbg", [QH, 128, 512], F32, kind="ExternalOutput")
        pt_dbg = nc.dram_tensor("pt_dbg", [4, 128, 512], BF16, kind="ExternalOutput")

    with tile.TileContext(nc) as tc:
        with (
            tc.tile_pool(name="const", bufs=1) as const,
            tc.tile_pool(name="wts", bufs=1) as wts,
            tc.tile_pool(name="wo", bufs=2) as wop,
            tc.tile_pool(name="xp", bufs=2) as xp,
            tc.tile_pool(name="cs", bufs=2) as csp,
            tc.tile_pool(name="qt", bufs=2) as qtp,
            tc.tile_pool(name="persist", bufs=1) as persist,
            tc.tile_pool(name="pt", bufs=1) as ptp,
            tc.tile_pool(name="rp", bufs=1) as rpp,
            tc.tile_pool(name="tt", bufs=1) as ttp,
            tc.tile_pool(name="dn", bufs=1) as dnp,
            tc.tile_pool(name="pr", bufs=1) as prp,
            tc.tile_pool(name="rb", bufs=1) as rbp,
            tc.tile_pool(name="at", bufs=3) as atp,
            tc.tile_pool(name="ob", bufs=2) as obp,
            tc.tile_pool(name="pj_psum", bufs=2, space="PSUM") as pj_psum,
            tc.tile_pool(name="sc_psum", bufs=2, space="PSUM") as sc_psum,
            tc.tile_pool(name="pv_psum", bufs=2, space="PSUM") as pv_psum,
            tc.tile_pool(name="op_psum", bufs=2, space="PSUM") as op_psum,
        ):
            # ---- persistent tensors ----
            KT = [persist.tile([128, L], BF16, tag=f"kt{g}", name=f"kt{g}")
                  for g in range(KVH)]
            VE = persist.tile([128, LT * 256], BF16, tag="ve", name="ve")
            QTS = [[None] * QH for _ in range(NQ)]
            maskt = []
            wk, wv, wq, xt = [], [], [], [None] * KC

            # ---- prologue DMA: first-needed first ----
            # K-proj of chunk 0 needs wk[k] + x0[k]; interleave so the PE can
            # start ~1us in and stream through the fill.
            for k in range(KC):
                w = wts.tile([128, KVH * 128], BF16, tag=f"k{k}", name=f"wk{k}")
                nc.sync.dma_start(w[:], wkt_d[k * 128:(k + 1) * 128, :])
                wk.append(w)
                t = xp.tile([128, 256], BF16, tag=f"x{k}", name=f"xc{k}")
                nc.sync.dma_start(t[:], xt_d[k * 128:(k + 1) * 128, 0:256])
                xt[k] = t
            for k in range(KC):
                w = wts.tile([128, QH * 128], BF16, tag=f"q{k}", name=f"wq{k}")
                nc.sync.dma_start(w[:], wqt_d[k * 128:(k + 1) * 128, :])
                wq.append(w)
            for k in range(KC):
                w = wts.tile([128, KVH * 128], BF16, tag=f"v{k}", name=f"wv{k}")
                nc.sync.dma_start(w[:], wvt_d[k * 128:(k + 1) * 128, :])
                wv.append(w)
            for o in range(4):
                m = const.tile([128, 512 - o * 128], BF16, tag=f"mask{o}")
                nc.sync.dma_start(m[:], masks_d[o][:, o * 128:512])
                maskt.append(m)

            def dma_cs(s2):
                cols = slice(s2 * 256, (s2 + 1) * 256)
                c = csp.tile([128, 256], BF16, tag="cos")
                nc.sync.dma_start(c[:], cost_d[:, cols])
                s = csp.tile([128, 256], BF16, tag="sin")
                nc.sync.dma_start(s[:], sint_d[:, cols])
                return c, s

            def dma_x(s2):
                xn = []
                for k in range(KC):
                    t = xp.tile([128, 256], BF16, tag=f"x{k}", name=f"xc{k}")
                    nc.sync.dma_start(
                        t[:], xt_d[k * 128:(k + 1) * 128,
                                   s2 * 256:(s2 + 1) * 256])
                    xn.append(t)
                return xn

            x_next = [list(xt)]

            def rope_drain(ps, dst, cosc, sinc):
                """psum [128,256] f32 -> rotate-half rope -> dst bf16."""
                t1 = rpp.tile([128, 256], F32, tag="r1")
                nc.vector.tensor_mul(t1[:], ps[:], cosc[:])
                t2 = rpp.tile([128, 256], F32, tag="r2")
                nc.vector.tensor_mul(t2[0:64, :], ps[64:128, :], sinc[0:64, :])
                nc.vector.tensor_mul(t2[64:128, :], ps[0:64, :], sinc[64:128, :])
                nc.vector.tensor_add(dst[:], t1[:], t2[:])

            def proj_sub(s2):
                """Projections for 256-col sub-chunk s2 (K, Q, V + rope).

                Prefetches sub-chunk s2+1's x tiles (bufs=2 ring, no WAR
                wait) so projection matmuls never stall on staging DMA.
                """
                nq, half = s2 // 2, s2 % 2
                xc = x_next[0]
                if s2 + 1 < 2 * NQ:
                    x_next[0] = dma_x(s2 + 1)
                cosc, sinc = dma_cs(s2)
                cols = slice(half * 256, half * 256 + 256)
                for g in range(KVH):
                    ps = pj_psum.tile([128, 256], F32, tag="pj")
                    for k in range(KC):
                        nc.tensor.matmul(
                            ps[:], wk[k][:, g * 128:(g + 1) * 128], xc[k][:],
                            start=(k == 0), stop=(k == KC - 1))
                    rope_drain(ps, KT[g][:, s2 * 256:(s2 + 1) * 256],
                               cosc, sinc)
                for h in range(QH):
                    if half == 0:
                        QTS[nq][h] = qtp.tile([128, 512], BF16, tag=f"qt{h}", name=f"qt{h}")
                    qt = QTS[nq][h]
                    ps = pj_psum.tile([128, 256], F32, tag="pj")
                    for k in range(KC):
                        nc.tensor.matmul(
                            ps[:], wq[k][:, h * 128:(h + 1) * 128], xc[k][:],
                            start=(k == 0), stop=(k == KC - 1))
                    rope_drain(ps, qt[:, cols], cosc, sinc)
                for b in range(2):
                    mk = s2 * 2 + b
                    ps = pj_psum.tile([128, 256], F32, tag="pj")
                    for k in range(KC):
                        nc.tensor.matmul(
                            ps[:], xc[k][:, b * 128:(b + 1) * 128],
                            wv[k][:], start=(k == 0), stop=(k == KC - 1))
                    nc.vector.tensor_copy(
                        VE[:, mk * 256:(mk + 1) * 256], ps[:])

            def proj_a(nq):
                proj_sub(2 * nq)

            def proj_b(nq):
                proj_sub(2 * nq + 1)

            def scores(nq, h):
                """scores -> tanh -> exp -> mask; GpSimd denom; rb recip."""
                g = h // 2
                nkt = 4 * nq + 4
                hp = h % 2
                pts = []
                dn = dnp.tile([128, 512], F32, tag="dn")
                for mk in range(nkt):
                    o = mk - 4 * nq
                    c0 = max(0, o) * 128
                    w = 512 - c0
                    ps_s = sc_psum.tile([128, 512], F32, tag="sc")
                    nc.tensor.matmul(
                        ps_s[:, 0:w], KT[g][:, mk * 128:(mk + 1) * 128],
                        QTS[nq][h][:, c0:512])
                    tt = ttp.tile([128, 512], F32, tag="tanh")
                    nc.scalar.activation(
                        tt[:, 0:w], ps_s[:, 0:w], AF.Tanh, scale=SCALE / SOFTCAP)
                    pt = ptp.tile([128, 512], BF16, tag=f"pt{hp}_{mk}")
                    pts.append(pt)
                    nc.scalar.activation(
                        pt[:, c0:512], tt[:, 0:w], AF.Exp, scale=SOFTCAP)
                    if o >= 0:
                        nc.vector.tensor_mul(
                            pt[:, c0:512], pt[:, c0:512], maskt[o][:, 0:w])
                    if mk == 0:
                        nc.gpsimd.tensor_copy(dn[:], pt[:])
                    else:
                        nc.gpsimd.tensor_add(
                            dn[:, c0:512], dn[:, c0:512], pt[:, c0:512])
                pr = prp.tile([128, 512], F32, tag="pr")
                nc.gpsimd.partition_all_reduce(
                    pr[:], dn[:], 128, bass_isa.ReduceOp.add)
                rb = rbp.tile([128, 512], F32, tag=f"rb{hp}")
                nc.vector.reciprocal_approx_fast(rb[:], pr[:])
                return rb, pts

            def pv(nq, h, rb, pts):
                """attn[d, q] = sum_mk V[mk]^T @ P^T[mk]; drain * recip."""
                g = h // 2
                nkt = 4 * nq + 4
                ps = pv_psum.tile([128, 512], F32, tag="pv")
                for mk in range(nkt):
                    o = mk - 4 * nq
                    c0 = max(0, o) * 128
                    pt = pts[mk]
                    nc.tensor.matmul(
                        ps[:, c0:512],
                        VE[:, mk * 256 + g * 128:mk * 256 + g * 128 + 128],
                        pt[:, c0:512],
                        start=(mk == 0), stop=(mk == nkt - 1))
                at = atp.tile([128, 512], BF16, tag=f"at{h}")
                nc.vector.tensor_mul(at[:], ps[:], rb[:])
                return at

            ATT = [[None] * QH for _ in range(NQ)]
            RB = [[None] * QH for _ in range(NQ)]

            def S(nq, h):
                RB[nq][h] = scores(nq, h)

            def V(nq, h):
                rb, pts = RB[nq][h]
                ATT[nq][h] = pv(nq, h, rb, pts)
                if DEBUG_TAPS and nq == 0:
                    nc.sync.dma_start(at_dbg[h], ATT[nq][h][:])
                    nc.sync.dma_start(rb_dbg[h], rb[:])
                    if h == 0:
                        for mk in range(4):
                            c0 = mk * 128
                            nc.sync.dma_start(
                                pt_dbg[mk][:, c0:512], pts[mk][:, c0:512])

            def dma_wo(j):
                woj = []
                for h in range(QH):
                    w = wop.tile([128, 512], BF16, tag=f"wo{h}", name=f"wo{h}")
                    nc.sync.dma_start(
                        w[:], wot_d[h * 128:(h + 1) * 128,
                                    j * 512:(j + 1) * 512])
                    woj.append(w)
                return woj

            def oproj(nq, j0, j1):
                """o_proj chunk nq for wo column-chunks j0..j1-1.

                wo tiles prefetched one j ahead so loads sit in front of the
                out-store DMAs in the SP queue.
                """
                wo_cur = dma_wo(j0)
                for j in range(j0, j1):
                    woj = wo_cur
                    if j + 1 < j1:
                        wo_cur = dma_wo(j + 1)
                    for s in range(4):
                        po = op_psum.tile([128, 512], F32, tag="op")
                        for h in range(QH):
                            nc.tensor.matmul(
                                po[:], ATT[nq][h][:, s * 128:(s + 1) * 128],
                                woj[h][:], start=(h == 0), stop=(h == QH - 1))
                        ob = obp.tile([128, 512], BF16, tag="ob")
                        nc.vector.tensor_copy(ob[:], po[:])
                        nc.sync.dma_start(
                            out_d[nq * 512 + s * 128:nq * 512 + (s + 1) * 128,
                                  j * 512:(j + 1) * 512], ob[:])

            # ---- software-pipelined schedule ----
            # Each slot pairs scalar-heavy score work with PE-heavy projection
            # or o_proj streams so tanh/exp always hides under matmuls.
            proj_a(0); proj_b(0)
            if DEBUG_TAPS:
                for h in range(QH):
                    nc.sync.dma_start(qt_dbg[h], QTS[0][h][:])
            S(0, 0); S(0, 1)
            proj_a(1)
            V(0, 0); S(0, 2)
            proj_b(1)
            V(0, 1); S(0, 3)
            proj_a(2)
            V(0, 2); S(1, 0)
            proj_b(2)
            V(0, 3); S(1, 1)
            oproj(0, 0, 5)
            V(1, 0); S(1, 2)
            oproj(0, 5, 9)
            V(1, 1); S(1, 3)
            proj_a(3)
            V(1, 2); S(2, 0)
            proj_b(3)
            V(1, 3); S(2, 1)
            oproj(1, 0, 5)
            V(2, 0); S(2, 2)
            oproj(1, 5, 9)
            V(2, 1); S(2, 3)
            V(2, 2); S(3, 0)
            V(2, 3); S(3, 1)
            oproj(2, 0, 5)
            V(3, 0); S(3, 2)
            oproj(2, 5, 9)
            V(3, 1); S(3, 3)
            V(3, 2)
            V(3, 3)
            oproj(3, 0, 9)
            if DEBUG_TAPS:
                for g in range(KVH):
                    nc.sync.dma_start(kt_dbg[g], KT[g][:])
                nc.sync.dma_start(ve_dbg[:], VE[:])
    return nc


_CACHED_NC = {}


def build():
    if 0 not in _CACHED_NC:
        nc = bacc.Bacc("TRN2", target_bir_lowering=False, debug=False)
        _emit(nc)
        nc.compile()
        _CACHED_NC[0] = nc
    return _CACHED_NC[0]


def host_tables():
    inv_freq = 1.0 / (ROPE_THETA ** (np.arange(0, HEAD_DIM, 2, dtype=np.float32) / HEAD_DIM))
    ang = np.arange(L, dtype=np.float32)[:, None] * inv_freq[None, :]  # [L, 64]
    cos, sin = np.cos(ang), np.sin(ang)
    cosT = np.concatenate([cos.T, cos.T], axis=0).astype(BF16_NP)
    sinT = np.concatenate([-sin.T, sin.T], axis=0).astype(BF16_NP)
    return np.ascontiguousarray(cosT), np.ascontiguousarray(sinT)


def host_masks():
    k = np.arange(128)[:, None]
    q = np.arange(512)[None, :]
    m = np.stack([(q >= k + 128 * o) for o in range(4)]).astype(BF16_NP)
    return np.ascontiguousarray(m)


def make_in_maps(x, wq, wk, wv, wo):
    cosT, sinT = host_tables()
    masks = host_masks()
    xt = np.ascontiguousarray(x.reshape(L, D).T).astype(BF16_NP)
    in_maps = []
    for c in range(N_CORES):
        qs = slice(c * QH * 128, (c + 1) * QH * 128)
        kvs = slice(c * KVH * 128, (c + 1) * KVH * 128)
        in_maps.append({
            "xt": xt,
            "wqt": np.ascontiguousarray(wq[qs].T.astype(BF16_NP)),
            "wkt": np.ascontiguousarray(wk[kvs].T.astype(BF16_NP)),
            "wvt": np.ascontiguousarray(wv[kvs].T.astype(BF16_NP)),
            "wot": np.ascontiguousarray(wo[:, qs].T.astype(BF16_NP)),
            "cost": cosT,
            "sint": sinT,
            "masks": masks,
        })
    return in_maps


def run(inputs, trace=False, trace_kwargs=None):
    from concourse.bass_utils import run_bass_kernel_spmd

    nc = build()
    x = np.asarray(inputs["x"], dtype=np.float32)
    in_maps = make_in_maps(
        x,
        np.asarray(inputs["wq"], dtype=np.float32),
        np.asarray(inputs["wk"], dtype=np.float32),
        np.asarray(inputs["wv"], dtype=np.float32),
        np.asarray(inputs["wo"], dtype=np.float32),
    )
    res = run_bass_kernel_spmd(
        nc, in_maps, core_ids=list(range(N_CORES)),
        trace=trace, **(trace_kwargs or {}))
    out = np.zeros((L, D), dtype=np.float32)
    for c in range(N_CORES):
        out += res.results[c]["out"].astype(np.float32)
    return out.reshape(x.shape), res


def kernel(**inputs) -> np.ndarray:
    out, _ = run(inputs, trace=False)
    return out


# revision 16
# speedup vs baseline: 1.1012x; 1.1012x over previous
"""Trainium2 Bass kernel for GQA attention (32 q heads / 16 kv heads, head_dim
128, L=2048, D=4608) with RoPE, tanh softcap 50, causal mask, o_proj.

Strategy: tensor-parallel over heads across 8 NeuronCores. Core c computes
q-heads 4c..4c+3 and kv-heads 2c..2c+1 end-to-end; the host sums the 8 partial
[L, D] outputs (bf16 partials, f32 host accumulation).

v2 design (vs the two-phase baseline):
  - single software-pipelined pass over the 4 q-chunks of 512: causality lets
    attention for chunk nq start right after its projections (K/V history for
    chunks <= nq is already computed), so the Scalar engine's tanh+exp stream
    (~200us) hides under the PE's projection matmuls instead of serializing a
    separate attention phase
  - PV computed in [d, q] layout (lhsT = V tile, rhs = P^T tile, 512-wide
    streams) so every PE matmul streams >= 256 columns and LDWEIGHTS stays
    shadow-loaded; this also eliminates the per-128-column PE transposes of
    the attention output (o_proj consumes [d, q] directly)
  - softmax denominator accumulated on the otherwise-idle GpSimd engine
    (tensor_add over P^T tiles + partition_all_reduce broadcast), reciprocal
    on DVE, folded into the PV psum drain multiply
  - rope drains moved off the Scalar engine: DVE multiplies read the
    projection psum directly (cos/sin mul + rotate-half add)
  - wq/wk/wv resident; wo streamed per (chunk, j) to fit SBUF; x staged per
    chunk; outputs written bf16
"""

import numpy as np
import ml_dtypes

import concourse.bass as bass
import concourse.mybir as mybir
import concourse.tile as tile
from concourse import bacc, bass_isa

F32 = mybir.dt.float32
BF16 = mybir.dt.bfloat16
BF16_NP = ml_dtypes.bfloat16
AF = mybir.ActivationFunctionType

N_HEADS = 32
N_KV = 16
HEAD_DIM = 128
ROPE_THETA = 10000.0
SOFTCAP = 50.0
SCALE = 1.0 / 12.0  # 1/sqrt(144)
L = 2048
D = 4608
N_CORES = 8
QH = N_HEADS // N_CORES        # 4 local q heads
KVH = N_KV // N_CORES          # 2 local kv heads
KC = D // 128                  # 36 contraction chunks
NQ = L // 512                  # 4 l-chunks of 512
LT = L // 128                  # 16 l-tiles of 128


DEBUG_TAPS = False


def _emit(nc):
    xt_d = nc.dram_tensor("xt", [KC, 128, L], BF16, kind="ExternalInput")
    wqt_d = nc.dram_tensor("wqt", [KC, 128, QH * 128], BF16, kind="ExternalInput")
    wkt_d = nc.dram_tensor("wkt", [KC, 128, KVH * 128], BF16, kind="ExternalInput")
    wvt_d = nc.dram_tensor("wvt", [KC, 128, KVH * 128], BF16, kind="ExternalInput")
    wot_d = nc.dram_tensor("wot", [QH, 128, D], BF16, kind="ExternalInput")
    cost_d = nc.dram_tensor("cost", [128, L], BF16, kind="ExternalInput")
    sint_d = nc.dram_tensor("sint", [128, L], BF16, kind="ExternalInput")
    masks_d = nc.dram_tensor("masks", [4, 128, 512], BF16, kind="ExternalInput")
    out_d = nc.dram_tensor("out", [NQ, 4, 128, D], BF16, kind="ExternalOutput")
    if DEBUG_TAPS:
        qt_dbg = nc.dram_tensor("qt_dbg", [QH, 128, 512], BF16, kind="ExternalOutput")
        kt_dbg = nc.dram_tensor("kt_dbg", [KVH, 128, L], BF16, kind="ExternalOutput")
        ve_dbg = nc.dram_tensor("ve_dbg", [128, LT * 256], BF16, kind="ExternalOutput")
        at_dbg = nc.dram_tensor("at_dbg", [QH, 128, 512], BF16, kind="ExternalOutput")
        rb_dbg = nc.dram_tensor("rb_dbg", [QH, 128, 512], F32, kind="ExternalOutput")
        pt_dbg = nc.dram_tensor("pt_dbg", [4, 128, 512], BF16, kind="ExternalOutput")

    with tile.TileContext(nc) as tc:
        with (
            tc.tile_pool(name="const", bufs=1) as const,
            tc.tile_pool(name="wts", bufs=1) as wts,
            tc.tile_pool(name="wo", bufs=2) as wop,
            tc.tile_pool(name="xp", bufs=2) as xp,
            tc.tile_pool(name="cs", bufs=2) as csp,
            tc.tile_pool(name="qt", bufs=2) as qtp,
            tc.tile_pool(name="persist", bufs=1) as persist,
            tc.tile_pool(name="pt", bufs=1) as ptp,
            tc.tile_pool(name="rp", bufs=1) as rpp,
            tc.tile_pool(name="tt", bufs=1) as ttp,
            tc.tile_pool(name="dn", bufs=1) as dnp,
            tc.tile_pool(name="rb", bufs=1) as rbp,
            tc.tile_pool(name="at", bufs=3) as atp,
            tc.tile_pool(name="ob", bufs=2) as obp,
            tc.tile_pool(name="pj_psum", bufs=2, space="PSUM") as pj_psum,
            tc.tile_pool(name="sc_psum", bufs=2, space="PSUM") as sc_psum,
            tc.tile_pool(name="pv_psum", bufs=2, space="PSUM") as pv_psum,
            tc.tile_pool(name="op_psum", bufs=2, space="PSUM") as op_psum,
        ):
            # ---- persistent tensors ----
            KT = [persist.tile([128, L], BF16, tag=f"kt{g}", name=f"kt{g}")
                  for g in range(KVH)]
            VE = persist.tile([128, LT * 256], BF16, tag="ve", name="ve")
            QTS = [[None] * QH for _ in range(NQ)]
            maskt = []

            # ---- prologue DMA: one descriptor per tensor (DMA-start pace
            # on the SP queue is ~1.8us each; count rules, not bytes) ----
            def dma_x(s2):
                t = xp.tile([128, KC * 256], BF16, tag="x", name="xc")
                nc.sync.dma_start(
                    t[:].rearrange("p (k c) -> p k c", k=KC),
                    xt_d[:, :, s2 * 256:(s2 + 1) * 256]
                    .rearrange("k p c -> p k c"))
                return t

            xc0 = dma_x(0)
            wk = wts.tile([128, KC * KVH * 128], BF16, tag="wk", name="wk")
            nc.sync.dma_start(
                wk[:].rearrange("p (k c) -> p k c", k=KC),
                wkt_d[:].rearrange("k p c -> p k c"))
            wq = wts.tile([128, KC * QH * 128], BF16, tag="wq", name="wq")
            nc.sync.dma_start(
                wq[:].rearrange("p (k c) -> p k c", k=KC),
                wqt_d[:].rearrange("k p c -> p k c"))
            wv = wts.tile([128, KC * KVH * 128], BF16, tag="wv", name="wv")
            nc.sync.dma_start(
                wv[:].rearrange("p (k c) -> p k c", k=KC),
                wvt_d[:].rearrange("k p c -> p k c"))
            for o in range(4):
                m = const.tile([128, 512 - o * 128], BF16, tag=f"mask{o}")
                nc.sync.dma_start(m[:], masks_d[o][:, o * 128:512])
                maskt.append(m)

            def dma_cs(s2):
                cols = slice(s2 * 256, (s2 + 1) * 256)
                c = csp.tile([128, 256], BF16, tag="cos")
                nc.sync.dma_start(c[:], cost_d[:, cols])
                s = csp.tile([128, 256], BF16, tag="sin")
                nc.sync.dma_start(s[:], sint_d[:, cols])
                return c, s

            x_next = [xc0]

            def rope_drain(ps, dst, cosc, sinc):
                """psum [128,256] f32 -> rotate-half rope -> dst bf16."""
                t1 = rpp.tile([128, 256], F32, tag="r1")
                nc.vector.tensor_mul(t1[:], ps[:], cosc[:])
                t2 = rpp.tile([128, 256], F32, tag="r2")
                nc.vector.tensor_mul(t2[0:64, :], ps[64:128, :], sinc[0:64, :])
                nc.vector.tensor_mul(t2[64:128, :], ps[0:64, :], sinc[64:128, :])
                nc.vector.tensor_add(dst[:], t1[:], t2[:])

            def proj_sub(s2):
                """Projections for 256-col sub-chunk s2 (K, Q, V + rope).

                Prefetches sub-chunk s2+1's x tiles (bufs=2 ring, no WAR
                wait) so projection matmuls never stall on staging DMA.
                """
                nq, half = s2 // 2, s2 % 2
                xc = x_next[0]
                if s2 + 1 < 2 * NQ:
                    x_next[0] = dma_x(s2 + 1)
                cosc, sinc = dma_cs(s2)
                cols = slice(half * 256, half * 256 + 256)
                for g in range(KVH):
                    ps = pj_psum.tile([128, 256], F32, tag="pj")
                    for k in range(KC):
                        nc.tensor.matmul(
                            ps[:], wk[:, k * 256 + g * 128:k * 256 + g * 128 + 128],
                            xc[:, k * 256:(k + 1) * 256],
                            start=(k == 0), stop=(k == KC - 1))
                    rope_drain(ps, KT[g][:, s2 * 256:(s2 + 1) * 256],
                               cosc, sinc)
                for h in range(QH):
                    if half == 0:
                        QTS[nq][h] = qtp.tile([128, 512], BF16, tag=f"qt{h}", name=f"qt{h}")
                    qt = QTS[nq][h]
                    ps = pj_psum.tile([128, 256], F32, tag="pj")
                    for k in range(KC):
                        nc.tensor.matmul(
                            ps[:], wq[:, k * 512 + h * 128:k * 512 + h * 128 + 128],
                            xc[:, k * 256:(k + 1) * 256],
                            start=(k == 0), stop=(k == KC - 1))
                    rope_drain(ps, qt[:, cols], cosc, sinc)
                for b in range(2):
                    mk = s2 * 2 + b
                    ps = pj_psum.tile([128, 256], F32, tag="pj")
                    for k in range(KC):
                        nc.tensor.matmul(
                            ps[:], xc[:, k * 256 + b * 128:k * 256 + b * 128 + 128],
                            wv[:, k * 256:(k + 1) * 256],
                            start=(k == 0), stop=(k == KC - 1))
                    nc.vector.tensor_copy(
                        VE[:, mk * 256:(mk + 1) * 256], ps[:])

            def proj_a(nq):
                proj_sub(2 * nq)

            def proj_b(nq):
                proj_sub(2 * nq + 1)

            def scores(nq, h):
                """scores -> tanh -> exp -> mask; GpSimd denom; rb recip."""
                g = h // 2
                nkt = 4 * nq + 4
                hp = h % 2
                pts = []
                dn = dnp.tile([128, 512], F32, tag="dn")
                for mk in range(nkt):
                    o = mk - 4 * nq
                    c0 = max(0, o) * 128
                    w = 512 - c0
                    ps_s = sc_psum.tile([128, 512], F32, tag="sc")
                    nc.tensor.matmul(
                        ps_s[:, 0:w], KT[g][:, mk * 128:(mk + 1) * 128],
                        QTS[nq][h][:, c0:512])
                    tt = ttp.tile([128, 512], F32, tag="tanh")
                    nc.scalar.activation(
                        tt[:, 0:w], ps_s[:, 0:w], AF.Tanh, scale=SCALE / SOFTCAP)
                    pt = ptp.tile([128, 512], BF16, tag=f"pt{hp}_{mk}")
                    pts.append(pt)
                    nc.scalar.activation(
                        pt[:, c0:512], tt[:, 0:w], AF.Exp, scale=SOFTCAP)
                    if o >= 0:
                        nc.vector.tensor_mul(
                            pt[:, c0:512], pt[:, c0:512], maskt[o][:, 0:w])
                    if mk == 0:
                        nc.gpsimd.tensor_copy(dn[:], pt[:])
                    else:
                        nc.gpsimd.tensor_add(
                            dn[:, c0:512], dn[:, c0:512], pt[:, c0:512])
                rb = rbp.tile([128, 512], F32, tag=f"rb{hp}")
                nc.gpsimd.partition_all_reduce(
                    rb[:], dn[:], 128, bass_isa.ReduceOp.add)
                nc.vector.reciprocal_approx_fast(rb[:], rb[:])
                return rb, pts

            def pv(nq, h, rb, pts):
                """attn[d, q] = sum_mk V[mk]^T @ P^T[mk]; drain * recip."""
                g = h // 2
                nkt = 4 * nq + 4
                ps = pv_psum.tile([128, 512], F32, tag="pv")
                for mk in range(nkt):
                    o = mk - 4 * nq
                    c0 = max(0, o) * 128
                    pt = pts[mk]
                    nc.tensor.matmul(
                        ps[:, c0:512],
                        VE[:, mk * 256 + g * 128:mk * 256 + g * 128 + 128],
                        pt[:, c0:512],
                        start=(mk == 0), stop=(mk == nkt - 1))
                at = atp.tile([128, 512], BF16, tag=f"at{h}")
                nc.vector.tensor_mul(at[:], ps[:], rb[:])
                return at

            ATT = [[None] * QH for _ in range(NQ)]
            RB = [[None] * QH for _ in range(NQ)]

            def S(nq, h):
                RB[nq][h] = scores(nq, h)

            def V(nq, h):
                rb, pts = RB[nq][h]
                ATT[nq][h] = pv(nq, h, rb, pts)
                if DEBUG_TAPS and nq == 0:
                    nc.sync.dma_start(at_dbg[h], ATT[nq][h][:])
                    nc.sync.dma_start(rb_dbg[h], rb[:])
                    if h == 0:
                        for mk in range(4):
                            c0 = mk * 128
                            nc.sync.dma_start(
                                pt_dbg[mk][:, c0:512], pts[mk][:, c0:512])

            def dma_wo(j):
                w = wop.tile([128, QH * 512], BF16, tag="wo", name="woj")
                nc.sync.dma_start(
                    w[:].rearrange("p (h c) -> p h c", h=QH),
                    wot_d[:, :, j * 512:(j + 1) * 512]
                    .rearrange("h p c -> p h c"))
                return w

            def oproj(nq, j0, j1):
                """o_proj chunk nq for wo column-chunks j0..j1-1.

                wo tiles prefetched one j ahead so loads sit in front of the
                out-store DMAs in the SP queue.
                """
                wo_cur = dma_wo(j0)
                for j in range(j0, j1):
                    woj = wo_cur
                    if j + 1 < j1:
                        wo_cur = dma_wo(j + 1)
                    ob = obp.tile([128, 4 * 512], BF16, tag="ob", name="ob4")
                    for s in range(4):
                        po = op_psum.tile([128, 512], F32, tag="op")
                        for h in range(QH):
                            nc.tensor.matmul(
                                po[:], ATT[nq][h][:, s * 128:(s + 1) * 128],
                                woj[:, h * 512:(h + 1) * 512],
                                start=(h == 0), stop=(h == QH - 1))
                        nc.vector.tensor_copy(ob[:, s * 512:(s + 1) * 512], po[:])
                    nc.sync.dma_start(
                        out_d[nq, :, :, j * 512:(j + 1) * 512]
                        .rearrange("s p c -> p s c"),
                        ob[:].rearrange("p (s c) -> p s c", s=4))

            # ---- software-pipelined schedule ----
            # Each slot pairs scalar-heavy score work with PE-heavy projection
            # or o_proj streams so tanh/exp always hides under matmuls.
            proj_a(0); proj_b(0)
            if DEBUG_TAPS:
                for h in range(QH):
                    nc.sync.dma_start(qt_dbg[h], QTS[0][h][:])
            S(0, 0); S(0, 1)
            proj_a(1)
            V(0, 0); S(0, 2)
            proj_b(1)
            V(0, 1); S(0, 3)
            proj_a(2)
            V(0, 2); S(1, 0)
            proj_b(2)
            V(0, 3); S(1, 1)
            oproj(0, 0, 5)
            V(1, 0); S(1, 2)
            oproj(0, 5, 9)
            V(1, 1); S(1, 3)
            proj_a(3)
            V(1, 2); S(2, 0)
            proj_b(3)
            V(1, 3); S(2, 1)
            oproj(1, 0, 5)
            V(2, 0); S(2, 2)
            oproj(1, 5, 9)
            V(2, 1); S(2, 3)
            V(2, 2); S(3, 0)
            V(2, 3); S(3, 1)
            oproj(2, 0, 5)
            V(3, 0); S(3, 2)
            oproj(2, 5, 9)
            V(3, 1); S(3, 3)
            V(3, 2)
            V(3, 3)
            oproj(3, 0, 9)
            if DEBUG_TAPS:
                for g in range(KVH):
                    nc.sync.dma_start(kt_dbg[g], KT[g][:])
                nc.sync.dma_start(ve_dbg[:], VE[:])
    return nc


_CACHED_NC = {}


def build():
    if 0 not in _CACHED_NC:
        nc = bacc.Bacc("TRN2", target_bir_lowering=False, debug=False)
        _emit(nc)
        nc.compile()
        _CACHED_NC[0] = nc
    return _CACHED_NC[0]


def host_tables():
    inv_freq = 1.0 / (ROPE_THETA ** (np.arange(0, HEAD_DIM, 2, dtype=np.float32) / HEAD_DIM))
    ang = np.arange(L, dtype=np.float32)[:, None] * inv_freq[None, :]  # [L, 64]
    cos, sin = np.cos(ang), np.sin(ang)
    cosT = np.concatenate([cos.T, cos.T], axis=0).astype(BF16_NP)
    sinT = np.concatenate([-sin.T, sin.T], axis=0).astype(BF16_NP)
    return np.ascontiguousarray(cosT), np.ascontiguousarray(sinT)


def host_masks():
    k = np.arange(128)[:, None]
    q = np.arange(512)[None, :]
    m = np.stack([(q >= k + 128 * o) for o in range(4)]).astype(BF16_NP)
    return np.ascontiguousarray(m)


def make_in_maps(x, wq, wk, wv, wo):
    cosT, sinT = host_tables()
    masks = host_masks()
    xt = np.ascontiguousarray(x.reshape(L, D).T).astype(BF16_NP).reshape(KC, 128, L)
    in_maps = []
    for c in range(N_CORES):
        qs = slice(c * QH * 128, (c + 1) * QH * 128)
        kvs = slice(c * KVH * 128, (c + 1) * KVH * 128)
        in_maps.append({
            "xt": xt,
            "wqt": np.ascontiguousarray(wq[qs].T.astype(BF16_NP)).reshape(KC, 128, QH * 128),
            "wkt": np.ascontiguousarray(wk[kvs].T.astype(BF16_NP)).reshape(KC, 128, KVH * 128),
            "wvt": np.ascontiguousarray(wv[kvs].T.astype(BF16_NP)).reshape(KC, 128, KVH * 128),
            "wot": np.ascontiguousarray(wo[:, qs].T.astype(BF16_NP)).reshape(QH, 128, D),
            "cost": cosT,
            "sint": sinT,
            "masks": masks,
        })
    return in_maps


def run(inputs, trace=False, trace_kwargs=None):
    from concourse.bass_utils import run_bass_kernel_spmd

    nc = build()
    x = np.asarray(inputs["x"], dtype=np.float32)
    in_maps = make_in_maps(
        x,
        np.asarray(inputs["wq"], dtype=np.float32),
        np.asarray(inputs["wk"], dtype=np.float32),
        np.asarray(inputs["wv"], dtype=np.float32),
        np.asarray(inputs["wo"], dtype=np.float32),
    )
    res = run_bass_kernel_spmd(
        nc, in_maps, core_ids=list(range(N_CORES)),
        trace=trace, **(trace_kwargs or {}))
    out = np.zeros((L, D), dtype=np.float32)
    for c in range(N_CORES):
        out += res.results[c]["out"].reshape(L, D).astype(np.float32)
    return out.reshape(x.shape), res


def kernel(**inputs) -> np.ndarray:
    out, _ = run(inputs, trace=False)
    return out
